# revision 5
# baseline (speedup 1.0000x reference)
"""Causal single-head attention on 8 trn2 NeuronCores.

Problem: x[4, 2048, 1024] fp32, W_q/W_k/W_v [1024, 1024] fp32 (torch Linear
layout, y = x @ W.T). Causal softmax attention, d_out = 1024.

Sharding: data-parallel over batch (4) x 2-way causal-balanced query split.
Core c = 2*b + h handles batch b and the four 256-row query blocks
{h, h+2, h+4, h+6} (global 256-row block indices). Interleaving blocks gives
both cores of a pair the same causal work profile, so one SPMD program with a
static key-extent schedule (superblock j attends keys [0, 512*(j+1))) fits
all cores; the causal boundary is handled by a per-core additive mask that
depends only on h and is supplied as data.

Per-core device program (all matmul operands bf16, fp32 PSUM accumulate):
  1. Projections: K^T[o, sk], V[sk, o], Q^T[o, sq] from x^T / xq^T and W^T.
  2. Flash-style attention per 256-row superblock with transposed scores
     S^T[sk, sq] (softmax denominator via PE ones-matmul), no max
     subtraction (scores bounded ~|2.5|), unnormalized AV accumulation in
     PSUM, final normalize by reciprocal of the denominator.
"""

import copy

import numpy as np
import ml_dtypes

import concourse.bass as bass
import concourse.mybir as mybir
import concourse.tile as tile
from concourse.bass_utils import run_bass_kernel_spmd

BF16 = mybir.dt.bfloat16
F32 = mybir.dt.float32

B, S, D = 4, 2048, 1024
N_CORES = 8
SB = 256          # query superblock rows
N_SB = 4          # superblocks per core (1024 query rows)
KEXT = 512        # key-extent step per superblock
MASK_NEG = -1.0e5


def _legalize_waits(nc):
    """Split multi-wait instructions into single-wait NOP chains.

    The walrus here accepts at most one sync-wait command per instruction,
    while TileContext emits several `on_wait` entries on one instruction.
    Hoist all but the last wait onto same-engine NOPs placed immediately
    before the instruction; the engine sequencer stalls on each in order.
    """
    uid = 0
    for fn in nc.m.functions:
        for bb in fn.blocks:
            out = []
            for inst in bb.instructions:
                si = inst.sync_info
                waits = list(si.on_wait) if si and si.on_wait else []
                if len(waits) > 1:
                    for w in waits[:-1]:
                        nop = mybir.InstNoOp(name=f"waitsplit_{uid}", ins=[], outs=[])
                        uid += 1
                        nop.engine = inst.engine
                        si2 = copy.deepcopy(si)
                        si2.on_wait = [w]
                        si2.on_update = []
                        nop.sync_info = si2
                        out.append(nop)
                    si.on_wait = waits[-1:]
                    inst.sync_info = si
                out.append(inst)
            bb.instructions = out


def build_nc():
    nc = bass.Bass("TRN2", target_bir_lowering=False, debug=False, num_devices=N_CORES)

    xT_d = nc.dram_tensor("xT", [D, S], BF16, kind="ExternalInput")
    xqT_d = nc.dram_tensor("xqT", [D, 1024], BF16, kind="ExternalInput")
    wqT_d = nc.dram_tensor("wqT", [D, D], BF16, kind="ExternalInput")
    wkT_d = nc.dram_tensor("wkT", [D, D], BF16, kind="ExternalInput")
    wvT_d = nc.dram_tensor("wvT", [D, D], BF16, kind="ExternalInput")
    maskT_d = nc.dram_tensor("maskT", [2 * SB, SB], F32, kind="ExternalInput")
    y_d = nc.dram_tensor("y", [1024, D], F32, kind="ExternalOutput")

    ND = D // 128      # 8 d-tiles
    NO = D // 128      # 8 o-tiles
    NSK = S // 128     # 16 key tiles

    with tile.TileContext(nc) as tc:
        with (
            tc.tile_pool(name="xT", bufs=ND) as xT_pool,
            tc.tile_pool(name="xqT", bufs=ND) as xqT_pool,
            tc.tile_pool(name="w", bufs=2 * ND) as w_pool,
            tc.tile_pool(name="KT", bufs=NO) as KT_pool,
            tc.tile_pool(name="V", bufs=NSK) as V_pool,
            tc.tile_pool(name="QT", bufs=NO) as QT_pool,
            tc.tile_pool(name="mask", bufs=4) as mask_pool,
            tc.tile_pool(name="ones", bufs=1) as ones_pool,
            tc.tile_pool(name="es", bufs=3) as es_pool,
            tc.tile_pool(name="inv", bufs=2) as inv_pool,
            tc.tile_pool(name="out", bufs=2) as out_pool,
        ):
            # ---- loads (emission order ~ consumption order) ----
            wk_t = []
            for i in range(ND):
                t = w_pool.tile([128, D], BF16, tag="w")
                nc.sync.dma_start(t[:], wkT_d[i * 128:(i + 1) * 128, :])
                wk_t.append(t)
            xT_t = []
            for i in range(ND):
                t = xT_pool.tile([128, S], BF16)
                nc.sync.dma_start(t[:], xT_d[i * 128:(i + 1) * 128, :])
                xT_t.append(t)
            wv_t = []
            for i in range(ND):
                t = w_pool.tile([128, D], BF16, tag="w")
                nc.sync.dma_start(t[:], wvT_d[i * 128:(i + 1) * 128, :])
                wv_t.append(t)
            wq_t = []
            for i in range(ND):
                t = w_pool.tile([128, D], BF16, tag="w")
                nc.sync.dma_start(t[:], wqT_d[i * 128:(i + 1) * 128, :])
                wq_t.append(t)
            xq_t = []
            for i in range(ND):
                t = xqT_pool.tile([128, 1024], BF16)
                nc.sync.dma_start(t[:], xqT_d[i * 128:(i + 1) * 128, :])
                xq_t.append(t)
            mask_t = []
            for i in range(4):
                t = mask_pool.tile([128, SB], F32)
                nc.sync.dma_start(t[:], maskT_d[i * 128:(i + 1) * 128, :])
                mask_t.append(t)
            ones_t = ones_pool.tile([128, 1], BF16)
            nc.vector.memset(ones_t[:], 1.0)

            # ---- phase 1: projections ----
            with tc.tile_pool(name="psum1", bufs=4, space="PSUM") as psum1:
                # K^T[o, sk] = sum_d wkT[d, o] * xT[d, sk]
                KT_t = []
                for ot in range(NO):
                    kt = KT_pool.tile([128, S], BF16)
                    KT_t.append(kt)
                    for skc in range(S // 512):
                        ps = psum1.tile([128, 512], F32, tag="ps1")
                        for d in range(ND):
                            nc.tensor.matmul(
                                ps[:],
                                wk_t[d][:, ot * 128:(ot + 1) * 128],
                                xT_t[d][:, skc * 512:(skc + 1) * 512],
                                start=(d == 0),
                                stop=(d == ND - 1),
                            )
                        nc.vector.tensor_copy(kt[:, skc * 512:(skc + 1) * 512], ps[:])
                # V[sk, o] = sum_d xT[d, sk] * wvT[d, o]
                V_t = []
                for st in range(NSK):
                    vt = V_pool.tile([128, D], BF16)
                    V_t.append(vt)
                    for oc in range(D // 512):
                        ps = psum1.tile([128, 512], F32, tag="ps1")
                        for d in range(ND):
                            nc.tensor.matmul(
                                ps[:],
                                xT_t[d][:, st * 128:(st + 1) * 128],
                                wv_t[d][:, oc * 512:(oc + 1) * 512],
                                start=(d == 0),
                                stop=(d == ND - 1),
                            )
                        nc.vector.tensor_copy(vt[:, oc * 512:(oc + 1) * 512], ps[:])
                # Q^T[o, sq] = sum_d wqT[d, o] * xqT[d, sq]
                QT_t = []
                for ot in range(NO):
                    qt = QT_pool.tile([128, 1024], BF16)
                    QT_t.append(qt)
                    for sqc in range(1024 // 512):
                        ps = psum1.tile([128, 512], F32, tag="ps1")
                        for d in range(ND):
                            nc.tensor.matmul(
                                ps[:],
                                wq_t[d][:, ot * 128:(ot + 1) * 128],
                                xq_t[d][:, sqc * 512:(sqc + 1) * 512],
                                start=(d == 0),
                                stop=(d == ND - 1),
                            )
                        nc.vector.tensor_copy(qt[:, sqc * 512:(sqc + 1) * 512], ps[:])

            # ---- phase 2: attention ----
            with (
                tc.tile_pool(name="av", bufs=4, space="PSUM") as av_pool,
                tc.tile_pool(name="pss", bufs=2, space="PSUM") as pss_pool,
                tc.tile_pool(name="den", bufs=2, space="PSUM") as den_pool,
            ):
                for j in range(N_SB):
                    n = (KEXT // 128) * (j + 1)  # sk-tiles this superblock
                    avs = [
                        av_pool.tile([128, 512], F32, tag="av", name=f"av{j}_{i}")
                        for i in range(4)
                    ]
                    dens = [
                        den_pool.tile([128, 1], F32, tag="den", name=f"den{j}_{i}")
                        for i in range(2)
                    ]
                    for t in range(n):
                        ps_s = pss_pool.tile([128, SB], F32, tag="pss")
                        for od in range(NO):
                            nc.tensor.matmul(
                                ps_s[:],
                                KT_t[od][:, t * 128:(t + 1) * 128],
                                QT_t[od][:, j * SB:(j + 1) * SB],
                                start=(od == 0),
                                stop=(od == NO - 1),
                            )
                        if t >= n - 4:
                            nc.vector.tensor_add(ps_s[:], ps_s[:], mask_t[t - (n - 4)][:])
                        es = es_pool.tile([128, SB], BF16, tag="es")
                        nc.scalar.activation(
                            es[:], ps_s[:], mybir.ActivationFunctionType.Exp,
                            scale=1.0 / 32.0,
                        )
                        first = (t == 0)
                        last = (t == n - 1)
                        for sqh in range(2):
                            for oh in range(2):
                                nc.tensor.matmul(
                                    avs[sqh * 2 + oh][:],
                                    es[:, sqh * 128:(sqh + 1) * 128],
                                    V_t[t][:, oh * 512:(oh + 1) * 512],
                                    start=first,
                                    stop=last,
                                )
                            nc.tensor.matmul(
                                dens[sqh][:],
                                es[:, sqh * 128:(sqh + 1) * 128],
                                ones_t[:],
                                start=first,
                                stop=last,
                            )
                    for sqh in range(2):
                        inv = inv_pool.tile([128, 1], F32, tag="inv")
                        nc.vector.reciprocal(inv[:], dens[sqh][:])
                        outt = out_pool.tile([128, D], F32, tag="out")
                        for oh in range(2):
                            nc.vector.tensor_scalar_mul(
                                outt[:, oh * 512:(oh + 1) * 512],
                                avs[sqh * 2 + oh][:],
                                inv[:],
                            )
                        r0 = j * SB + sqh * 128
                        nc.sync.dma_start(y_d[r0:r0 + 128, :], outt[:])

    _legalize_waits(nc)
    return nc


_NC_CACHE = None


def _get_nc():
    global _NC_CACHE
    if _NC_CACHE is None:
        _NC_CACHE = build_nc()
    return _NC_CACHE


def _prep_core_inputs(x, wqT, wkT, wvT, b, h):
    xb = np.ascontiguousarray(x[b])                       # [S, D] fp32
    xT = np.ascontiguousarray(xb.T).astype(ml_dtypes.bfloat16)
    blocks = [h + 2 * i for i in range(N_SB)]
    xq = np.concatenate([xb[SB * t:SB * (t + 1)] for t in blocks], axis=0)
    xqT = np.ascontiguousarray(xq.T).astype(ml_dtypes.bfloat16)
    cc = np.arange(2 * SB)[:, None]
    rr = np.arange(SB)[None, :]
    maskT = np.where(cc <= SB * h + rr, 0.0, MASK_NEG).astype(np.float32)
    return {
        "xT": xT, "xqT": xqT,
        "wqT": wqT, "wkT": wkT, "wvT": wvT,
        "maskT": maskT,
    }


def kernel(x, W_q, W_k, W_v):
    x = np.asarray(x, dtype=np.float32)
    wqT = np.ascontiguousarray(np.asarray(W_q, np.float32).T).astype(ml_dtypes.bfloat16)
    wkT = np.ascontiguousarray(np.asarray(W_k, np.float32).T).astype(ml_dtypes.bfloat16)
    wvT = np.ascontiguousarray(np.asarray(W_v, np.float32).T).astype(ml_dtypes.bfloat16)

    in_maps = []
    for c in range(N_CORES):
        b, h = divmod(c, 2)
        in_maps.append(_prep_core_inputs(x, wqT, wkT, wvT, b, h))

    nc = _get_nc()
    res = run_bass_kernel_spmd(nc, in_maps, list(range(N_CORES)))

    out = np.empty((B, S, D), dtype=np.float32)
    for c in range(N_CORES):
        b, h = divmod(c, 2)
        y = res.results[c]["y"]
        for ji in range(N_SB):
            t = h + 2 * ji
            out[b, SB * t:SB * (t + 1)] = y[SB * ji:SB * (ji + 1)]
    return out
